# revision 16
# baseline (speedup 1.0000x reference)
"""DDALoss Trainium2 kernel (8 NeuronCores, class-sharded).

Math (identical to the reference up to fp8/poly noise):
  lse[n]  = log(sum_c exp(2*feat[n]@centers[c] - ||c||^2))
          = log(sum_c w_c * exp(2*cross_nc)),  w_c = exp(-csq_c)
  nll_sum = sum_n (lse[n] - glab[n]);  glab/centerloss computed on host (fp64).

Per-core schedule (class shard: 1280 classes x all 4096 rows, [c, n] PSUM):
  - PE: psum[c128, n512] = fp8 DoubleRow cross matmul (4 K-blocks, 2 passes)
    -- the only irreducible device work (fp8 peak), ~34 us/core.
  - exp lanes: ACT (native Exp, 1024-wide psum->fp8 sbuf) alternating with the
    single-pass custom DVE op EXPQ16_ANT: (p2(x/16))^16 via 4 Horner stages +
    4 squarings, psum fp32 -> fp8.
  - E tiles stream back to DRAM (5.24 MB/core, overlapped with compute); the
    w-weighted class reduction (0.01% of FLOPs) and log/combine run on host.
  - All DMA triggers issue from the otherwise-idle GpSimd queue (25 ns each
    vs 667 ns on SP), so the first matmul starts ~2 us in.
"""

import sys

sys.path.insert(0, "/opt/trn_rl_repo")

import numpy as np
import ml_dtypes

from contextlib import ExitStack

import concourse.bass as bass
import concourse.bacc as bacc
import concourse.tile as tile
from concourse import mybir

# Problem constants (hardcoded per harness contract)
N = 4096
D = 512
C = 10000
CP = 10240  # classes padded to 8*1280
NCORES = 8
CPC = CP // NCORES  # 1280 classes per core
CT = CPC // 128  # 10 class tiles per core
NB = N // 512  # 8 batch blocks of 512
NGRP = CT // 2  # 5 groups of 2 class tiles

LAMB = 0.01
GAMMA = 3.0

FP8 = mybir.dt.float8e4
F32 = mybir.dt.float32

# fp8 scaling: psum = FS*CS*cross; exp arg x = psum/(FS*CS/2) = psum * ASC
FS = 8.0
CS = 16.0
ASC = 2.0 / (FS * CS)  # 1/64

_CACHE = {}

# ---- custom DVE exp op ----------------------------------------------------
# p2 relative-minimax of e^y on y in [-0.285, 0.285]; exp(x) ~= p2(x/16)^16.
# Coefficients fold in the /16 range reduction, the psum scale ASC, and a
# global bias correction that zeroes the expected weighted-sum error for
# x ~ N(0, 0.65^2) importance-weighted by e^x.
_P2 = (1.00020371, 1.01007938, 0.49746446)  # c0 + c1 y + c2 y^2


def _register_expq16():
    import concourse.dve_ops as dops
    from concourse.dve_spec import Spec, Src0, C0, C1, C2, sq, lower
    from concourse.dve_spec import _has_src1
    from concourse.dve_uop import DveOpSpec

    if "EXPQ16_ANT" in dops._SUB_OPCODE_FOR_NAME:
        return dops._EXPQ16_ANT  # (op, c2, c1, c0)

    # bias correction: divide poly by (1+b)^(1/16)
    bias = 0.0066386
    k = (1.0 / (1.0 + bias)) ** (1.0 / 16.0)
    s = ASC / 16.0  # psum -> y
    c0 = _P2[0] * k
    c1 = _P2[1] * k * s
    c2 = _P2[2] * k * s * s

    # body = sq^4((C0*g + C1)*g + C2): C0=c2, C1=c1, C2=c0
    body = sq(sq(sq(sq((Src0 * C0 + C1) * Src0 + C2))))

    def _ref(in0, in1, s0, s1, imm2):
        g = in0.astype(np.float32)
        p = (g * s0 + s1) * g + imm2
        return (((p * p) ** 2) ** 2) ** 2

    spec = Spec(body=body, reference=_ref)
    op = dops.DveOp("EXPQ16_ANT", spec, subdim=False, uops_sha={})
    dops.OPS.append(op)
    dops.CUSTOM_DVE_SPECS[op.name] = op.spec
    dops._SUB_OPCODE_FOR_NAME[op.name] = dops._CUSTOM_DVE_ROW_BASE + len(dops.OPS) - 1
    # pin the sha (computed, not hand-copied)
    tmp = DveOpSpec(
        name=op.name,
        opcode=dops.get_dve_sub_opcode(op.name),
        uops=lower(spec, ver="v3"),
        rd1_en=_has_src1(spec),
    )
    op.uops_sha["v3"] = tmp.sha("v3")
    dops._EXPQ16_ANT = (op, c2, c1, c0)
    return dops._EXPQ16_ANT


def _build():
    # (op, c2, c1, c0) mapped to custom-dve scalars (s0, s1, imm2)
    expq, cc0, cc1, cc2 = _register_expq16()

    nc = bacc.Bacc("TRN2", target_bir_lowering=False, debug=False)

    ctt = nc.dram_tensor("ctt", [D, CPC], FP8, kind="ExternalInput")  # centers.T slice
    ftt = nc.dram_tensor("ftt", [D, N], FP8, kind="ExternalInput")  # feat.T (full)
    # out[nb*CT*128 + t*128 + p, n'] = exp tile element (class t*128+p, col nb*512+n')
    out = nc.dram_tensor("out", [NB * CT * 128, 512], FP8, kind="ExternalOutput")

    ct_r = ctt.ap().rearrange("(k p) c -> p k c", p=128)  # [128, 4, CPC]
    ft_r = ftt.ap().rearrange("(k p) n -> p k n", p=128)  # [128, 4, N]
    out_r = out.ap().rearrange("(nb t p) n -> nb p t n", p=128, t=CT)  # [NB,128,CT,512]

    with tile.TileContext(nc) as tc, ExitStack() as ctx:
        const = ctx.enter_context(tc.tile_pool(name="const", bufs=1))
        ep = ctx.enter_context(tc.tile_pool(name="ep", bufs=3))
        pm = ctx.enter_context(tc.tile_pool(name="pm", bufs=4, space="PSUM"))

        ct_sb = const.tile([128, 4, CPC], FP8, tag="ct")
        ft_sb = const.tile([128, 4, N], FP8, tag="ft")
        # input DMAs on the two HWDGE queues (SP/ACT, ~0.6 us config each):
        # the first chunk of each lands by ~7 us; GpSimd SWDGE (~1 us serial
        # trigger each) is reserved for output drains where it's off-path
        nc.sync.dma_start(out=ct_sb[:, :, :256], in_=ct_r[:, :, :256])
        nc.scalar.dma_start(out=ft_sb[:, :, :512], in_=ft_r[:, :, :512])
        nc.sync.dma_start(out=ct_sb[:, :, 256:768], in_=ct_r[:, :, 256:768])
        nc.sync.dma_start(out=ct_sb[:, :, 768:], in_=ct_r[:, :, 768:])
        for nb in range(1, NB):
            s = slice(nb * 512, (nb + 1) * 512)
            nc.sync.dma_start(out=ft_sb[:, :, s], in_=ft_r[:, :, s])

        # PE p-state warmup: dummy passes during the ~10 us input-DMA wait so
        # the real matmuls start at full clock (memset on the idle DVE)
        wup = const.tile([128, 2, 512], FP8, tag="wup")
        nc.vector.memset(wup, 1.0)
        for _ in range(16):
            pw = pm.tile([128, 2, 512], F32, tag="pm")
            nc.tensor.matmul(
                out=pw[:, 0, :],
                lhsT=wup[:, :, :128],
                rhs=wup,
                start=True,
                stop=True,
                perf_mode=mybir.MatmulPerfMode.DoubleRow,
            )

        for nb in range(NB):
            ns = slice(nb * 512, (nb + 1) * 512)
            et = ep.tile([128, CT, 512], FP8, tag="et")
            for g in range(NGRP):
                pmt = pm.tile([128, 2, 512], F32, tag="pm")
                for half in range(2):
                    c0 = (g * 2 + half) * 128
                    for kp in range(2):
                        nc.tensor.matmul(
                            out=pmt[:, half, :],
                            lhsT=ct_sb[:, 2 * kp : 2 * kp + 2, c0 : c0 + 128],
                            rhs=ft_sb[:, 2 * kp : 2 * kp + 2, ns],
                            start=(kp == 0),
                            stop=(kp == 1),
                            perf_mode=mybir.MatmulPerfMode.DoubleRow,
                        )
                ets = et[:, 2 * g : 2 * g + 2, :]
                # alternate exp lanes; 3:2 / 2:3 by nb parity for balance,
                # except the final block ends on the faster ACT lane
                if nb == NB - 1:
                    use_act = g % 2 == 0
                else:
                    use_act = (g % 2 == 0) if nb % 2 == 0 else (g % 2 == 1)
                if use_act:
                    nc.scalar.activation(
                        ets, pmt, mybir.ActivationFunctionType.Exp, scale=ASC
                    )
                else:
                    nc.vector._custom_dve(
                        expq, out=ets, in0=pmt, s0=cc0, s1=cc1, imm2=cc2
                    )
                if nb == NB - 1:
                    # last block drains per-group on the (by now idle) SP
                    # queue -- the GpSimd SWDGE triggers queue ~1 us each
                    nc.sync.dma_start(
                        out=out_r[nb, :, 2 * g : 2 * g + 2, :], in_=ets
                    )
            if nb < NB - 1:
                nc.gpsimd.dma_start(out=out_r[nb], in_=et)

    nc.compile()
    return nc


def _get_nc():
    if "nc" not in _CACHE:
        _CACHE["nc"] = _build()
    return _CACHE["nc"]


def make_in_maps(feat, label, centers):
    feat = np.ascontiguousarray(np.asarray(feat, dtype=np.float32))
    centers = np.ascontiguousarray(np.asarray(centers, dtype=np.float32))
    label = np.asarray(label).astype(np.int64).reshape(N)

    f8 = ml_dtypes.float8_e4m3
    ftt = np.ascontiguousarray(feat.T * FS).astype(f8)  # [D, N]
    cT_pad = np.zeros((D, CP), dtype=f8)
    cT_pad[:, :C] = (centers.T * CS).astype(f8)

    c64 = centers.astype(np.float64)
    csq = (c64 * c64).sum(axis=1)  # [C]
    w_pad = np.zeros(CP, dtype=np.float64)
    w_pad[:C] = np.exp(-csq)

    # host-exact terms for the final combine
    f64 = feat.astype(np.float64)
    clab = c64[label]  # [N, D]
    diff = f64 - clab
    centerloss = (diff * diff).sum() / (2.0 * N)
    glab = 2.0 * (f64 * clab).sum(axis=1) - csq[label]
    _CACHE["host"] = (centerloss, glab, w_pad)

    in_maps = []
    for i in range(NCORES):
        sl = slice(i * CPC, (i + 1) * CPC)
        in_maps.append(
            {
                "ctt": np.ascontiguousarray(cT_pad[:, sl]),
                "ftt": ftt,
            }
        )
    return in_maps


# fp8 bits -> f32 lookup table for the fast host-side decode
_F8_LUT = (
    np.arange(256, dtype=np.uint8).view(ml_dtypes.float8_e4m3).astype(np.float32)
)


def combine(parts):
    centerloss, glab, w_pad = _CACHE["host"]
    S = np.zeros((NB, 512), dtype=np.float64)
    for i, p in enumerate(parts):
        raw = np.asarray(p).reshape(NB * CT * 128, 512)
        e32 = _F8_LUT[raw.view(np.uint8)].reshape(NB, CPC, 512)
        w = w_pad[i * CPC : (i + 1) * CPC].astype(np.float32)
        # S[nb, n'] += sum_c w_c * E[nb, c, n']
        S += np.einsum("bcn,c->bn", e32, w, optimize=True)
    lse = np.log(S.reshape(N))
    nll_sum = (lse - glab).sum()
    ddaloss = nll_sum / (2.0 * N * N)
    loss = LAMB * centerloss + GAMMA * ddaloss
    return loss, centerloss, ddaloss


def kernel(feat, label, centers):
    from concourse.bass_utils import run_bass_kernel_spmd

    in_maps = make_in_maps(feat, label, centers)
    nc = _get_nc()
    res = run_bass_kernel_spmd(nc, in_maps, core_ids=list(range(NCORES)))
    parts = [r["out"] for r in res.results]
    loss, centerloss, ddaloss = combine(parts)
    return (
        np.float32(loss),
        np.float32(centerloss),
        np.float32(ddaloss),
    )


# revision 17
# speedup vs baseline: 1.0281x; 1.0281x over previous
"""DDALoss Trainium2 kernel (8 NeuronCores, class-sharded).

Math (identical to the reference up to fp8/poly noise):
  lse[n]  = log(sum_c exp(2*feat[n]@centers[c] - ||c||^2))
          = log(sum_c w_c * exp(2*cross_nc)),  w_c = exp(-csq_c)
  nll_sum = sum_n (lse[n] - glab[n]);  glab/centerloss computed on host (fp64).

Per-core schedule (class shard: 1280 classes x all 4096 rows, [c, n] PSUM):
  - PE: psum[c128, n512] = fp8 DoubleRow cross matmul (4 K-blocks, 2 passes)
    -- the only irreducible device work (fp8 peak), ~34 us/core.
  - exp lanes: ACT (native Exp, 1024-wide psum->fp8 sbuf) alternating with the
    single-pass custom DVE op EXPQ16_ANT: (p2(x/16))^16 via 4 Horner stages +
    4 squarings, psum fp32 -> fp8.
  - E tiles stream back to DRAM (5.24 MB/core, overlapped with compute); the
    w-weighted class reduction (0.01% of FLOPs) and log/combine run on host.
  - All DMA triggers issue from the otherwise-idle GpSimd queue (25 ns each
    vs 667 ns on SP), so the first matmul starts ~2 us in.
"""

import sys

sys.path.insert(0, "/opt/trn_rl_repo")

import numpy as np
import ml_dtypes

from contextlib import ExitStack

import concourse.bass as bass
import concourse.bacc as bacc
import concourse.tile as tile
from concourse import mybir

# Problem constants (hardcoded per harness contract)
N = 4096
D = 512
C = 10000
CP = 10240  # classes padded to 8*1280
NCORES = 8
CPC = CP // NCORES  # 1280 classes per core
CT = CPC // 128  # 10 class tiles per core
NB = N // 512  # 8 batch blocks of 512
NGRP = CT // 2  # 5 groups of 2 class tiles

LAMB = 0.01
GAMMA = 3.0

FP8 = mybir.dt.float8e4
F32 = mybir.dt.float32

# fp8 scaling: psum = FS*CS*cross; exp arg x = psum/(FS*CS/2) = psum * ASC
FS = 8.0
CS = 16.0
ASC = 2.0 / (FS * CS)  # 1/64

_CACHE = {}

# ---- custom DVE exp op ----------------------------------------------------
# p2 relative-minimax of e^y on y in [-0.285, 0.285]; exp(x) ~= p2(x/16)^16.
# Coefficients fold in the /16 range reduction, the psum scale ASC, and a
# global bias correction that zeroes the expected weighted-sum error for
# x ~ N(0, 0.65^2) importance-weighted by e^x.
_P2 = (1.00020371, 1.01007938, 0.49746446)  # c0 + c1 y + c2 y^2


def _register_expq16():
    import concourse.dve_ops as dops
    from concourse.dve_spec import Spec, Src0, C0, C1, C2, sq, lower
    from concourse.dve_spec import _has_src1
    from concourse.dve_uop import DveOpSpec

    if "EXPQ16_ANT" in dops._SUB_OPCODE_FOR_NAME:
        return dops._EXPQ16_ANT  # (op, c2, c1, c0)

    # bias correction: divide poly by (1+b)^(1/16)
    bias = 0.0066386
    k = (1.0 / (1.0 + bias)) ** (1.0 / 16.0)
    s = ASC / 16.0  # psum -> y
    c0 = _P2[0] * k
    c1 = _P2[1] * k * s
    c2 = _P2[2] * k * s * s

    # body = sq^4((C0*g + C1)*g + C2): C0=c2, C1=c1, C2=c0
    body = sq(sq(sq(sq((Src0 * C0 + C1) * Src0 + C2))))

    def _ref(in0, in1, s0, s1, imm2):
        g = in0.astype(np.float32)
        p = (g * s0 + s1) * g + imm2
        return (((p * p) ** 2) ** 2) ** 2

    spec = Spec(body=body, reference=_ref)
    op = dops.DveOp("EXPQ16_ANT", spec, subdim=False, uops_sha={})
    dops.OPS.append(op)
    dops.CUSTOM_DVE_SPECS[op.name] = op.spec
    dops._SUB_OPCODE_FOR_NAME[op.name] = dops._CUSTOM_DVE_ROW_BASE + len(dops.OPS) - 1
    # pin the sha (computed, not hand-copied)
    tmp = DveOpSpec(
        name=op.name,
        opcode=dops.get_dve_sub_opcode(op.name),
        uops=lower(spec, ver="v3"),
        rd1_en=_has_src1(spec),
    )
    op.uops_sha["v3"] = tmp.sha("v3")
    dops._EXPQ16_ANT = (op, c2, c1, c0)
    return dops._EXPQ16_ANT


def _build():
    # (op, c2, c1, c0) mapped to custom-dve scalars (s0, s1, imm2)
    expq, cc0, cc1, cc2 = _register_expq16()

    nc = bacc.Bacc("TRN2", target_bir_lowering=False, debug=False)

    ctt = nc.dram_tensor("ctt", [D, CPC], FP8, kind="ExternalInput")  # centers.T slice
    ftt = nc.dram_tensor("ftt", [D, N], FP8, kind="ExternalInput")  # feat.T (full)
    # out[nb*CT*128 + t*128 + p, n'] = exp tile element (class t*128+p, col nb*512+n')
    out = nc.dram_tensor("out", [NB * CT * 128, 512], FP8, kind="ExternalOutput")

    ct_r = ctt.ap().rearrange("(k p) c -> p k c", p=128)  # [128, 4, CPC]
    ft_r = ftt.ap().rearrange("(k p) n -> p k n", p=128)  # [128, 4, N]
    out_r = out.ap().rearrange("(nb t p) n -> nb p t n", p=128, t=CT)  # [NB,128,CT,512]

    with tile.TileContext(nc) as tc, ExitStack() as ctx:
        const = ctx.enter_context(tc.tile_pool(name="const", bufs=1))
        ep = ctx.enter_context(tc.tile_pool(name="ep", bufs=3))
        pm = ctx.enter_context(tc.tile_pool(name="pm", bufs=4, space="PSUM"))

        ct_sb = const.tile([128, 4, CPC], FP8, tag="ct")
        ft_sb = const.tile([128, 4, N], FP8, tag="ft")
        # input DMAs on the two HWDGE queues (SP/ACT, ~0.6 us config each):
        # the first chunk of each lands by ~7 us; GpSimd SWDGE (~1 us serial
        # trigger each) is reserved for output drains where it's off-path
        nc.sync.dma_start(out=ct_sb[:, :, :256], in_=ct_r[:, :, :256])
        nc.scalar.dma_start(out=ft_sb[:, :, :512], in_=ft_r[:, :, :512])
        nc.sync.dma_start(out=ct_sb[:, :, 256:768], in_=ct_r[:, :, 256:768])
        nc.sync.dma_start(out=ct_sb[:, :, 768:], in_=ct_r[:, :, 768:])
        for nb in range(1, NB):
            s = slice(nb * 512, (nb + 1) * 512)
            nc.sync.dma_start(out=ft_sb[:, :, s], in_=ft_r[:, :, s])

        # PE p-state warmup: ~3 us of dummy passes covering the input-DMA
        # wait (data lands ~9.9 us; gpsimd memset is ready earliest, ~6.1 us)
        wup = const.tile([128, 2, 512], FP8, tag="wup")
        nc.gpsimd.memset(wup, 1.0)
        for _ in range(10):
            pw = pm.tile([128, 2, 512], F32, tag="pm")
            nc.tensor.matmul(
                out=pw[:, 0, :256],
                lhsT=wup[:, :, :128],
                rhs=wup[:, :, :256],
                start=True,
                stop=True,
                perf_mode=mybir.MatmulPerfMode.DoubleRow,
            )

        for nb in range(NB):
            ns = slice(nb * 512, (nb + 1) * 512)
            et = ep.tile([128, CT, 512], FP8, tag="et")
            for g in range(NGRP):
                pmt = pm.tile([128, 2, 512], F32, tag="pm")
                for half in range(2):
                    c0 = (g * 2 + half) * 128
                    for kp in range(2):
                        nc.tensor.matmul(
                            out=pmt[:, half, :],
                            lhsT=ct_sb[:, 2 * kp : 2 * kp + 2, c0 : c0 + 128],
                            rhs=ft_sb[:, 2 * kp : 2 * kp + 2, ns],
                            start=(kp == 0),
                            stop=(kp == 1),
                            perf_mode=mybir.MatmulPerfMode.DoubleRow,
                        )
                ets = et[:, 2 * g : 2 * g + 2, :]
                # alternate exp lanes; 3:2 / 2:3 by nb parity for balance,
                # except the final block ends on the faster ACT lane
                if nb == NB - 1:
                    use_act = g % 2 == 0
                else:
                    use_act = (g % 2 == 0) if nb % 2 == 0 else (g % 2 == 1)
                if use_act:
                    nc.scalar.activation(
                        ets, pmt, mybir.ActivationFunctionType.Exp, scale=ASC
                    )
                else:
                    nc.vector._custom_dve(
                        expq, out=ets, in0=pmt, s0=cc0, s1=cc1, imm2=cc2
                    )
                if nb == NB - 1:
                    # last block drains per-group on the (by now idle) SP
                    # queue -- the GpSimd SWDGE triggers queue ~1 us each
                    nc.sync.dma_start(
                        out=out_r[nb, :, 2 * g : 2 * g + 2, :], in_=ets
                    )
            if nb < NB - 1:
                nc.gpsimd.dma_start(out=out_r[nb], in_=et)

    nc.compile()
    return nc


def _get_nc():
    if "nc" not in _CACHE:
        _CACHE["nc"] = _build()
    return _CACHE["nc"]


def make_in_maps(feat, label, centers):
    feat = np.ascontiguousarray(np.asarray(feat, dtype=np.float32))
    centers = np.ascontiguousarray(np.asarray(centers, dtype=np.float32))
    label = np.asarray(label).astype(np.int64).reshape(N)

    f8 = ml_dtypes.float8_e4m3
    ftt = np.ascontiguousarray(feat.T * FS).astype(f8)  # [D, N]
    cT_pad = np.zeros((D, CP), dtype=f8)
    cT_pad[:, :C] = (centers.T * CS).astype(f8)

    c64 = centers.astype(np.float64)
    csq = (c64 * c64).sum(axis=1)  # [C]
    w_pad = np.zeros(CP, dtype=np.float64)
    w_pad[:C] = np.exp(-csq)

    # host-exact terms for the final combine
    f64 = feat.astype(np.float64)
    clab = c64[label]  # [N, D]
    diff = f64 - clab
    centerloss = (diff * diff).sum() / (2.0 * N)
    glab = 2.0 * (f64 * clab).sum(axis=1) - csq[label]
    _CACHE["host"] = (centerloss, glab, w_pad)

    in_maps = []
    for i in range(NCORES):
        sl = slice(i * CPC, (i + 1) * CPC)
        in_maps.append(
            {
                "ctt": np.ascontiguousarray(cT_pad[:, sl]),
                "ftt": ftt,
            }
        )
    return in_maps


# fp8 bits -> f32 lookup table for the fast host-side decode
_F8_LUT = (
    np.arange(256, dtype=np.uint8).view(ml_dtypes.float8_e4m3).astype(np.float32)
)


def combine(parts):
    centerloss, glab, w_pad = _CACHE["host"]
    S = np.zeros((NB, 512), dtype=np.float64)
    for i, p in enumerate(parts):
        raw = np.asarray(p).reshape(NB * CT * 128, 512)
        e32 = _F8_LUT[raw.view(np.uint8)].reshape(NB, CPC, 512)
        w = w_pad[i * CPC : (i + 1) * CPC].astype(np.float32)
        # S[nb, n'] += sum_c w_c * E[nb, c, n']
        S += np.einsum("bcn,c->bn", e32, w, optimize=True)
    lse = np.log(S.reshape(N))
    nll_sum = (lse - glab).sum()
    ddaloss = nll_sum / (2.0 * N * N)
    loss = LAMB * centerloss + GAMMA * ddaloss
    return loss, centerloss, ddaloss


def kernel(feat, label, centers):
    from concourse.bass_utils import run_bass_kernel_spmd

    in_maps = make_in_maps(feat, label, centers)
    nc = _get_nc()
    res = run_bass_kernel_spmd(nc, in_maps, core_ids=list(range(NCORES)))
    parts = [r["out"] for r in res.results]
    loss, centerloss, ddaloss = combine(parts)
    return (
        np.float32(loss),
        np.float32(centerloss),
        np.float32(ddaloss),
    )
